# revision 20
# baseline (speedup 1.0000x reference)
"""Trainium2 Bass kernel for a Dango-like HyperSAGNN block.

Reference computation (fp32):
  static = relu(X @ Ws.T + bs)                         # (32768, 768)
  x = X
  for l in 0..1:
      q/k/v = x @ W{q,k,v}[l].T + b{q,k,v}[l]          # per-group (16) masked MHA
      attn  = softmax over in-group, non-self keys
      out   = (attn @ v) @ Wo[l].T + bo[l]
      x     = x + beta[l] * out
  returns (static, x)

Strategy: data-parallel over groups — 8 cores x 4096 genes (256 groups).
Activations are kept feature-major ([768, genes]) in SBUF so every
projection is a dense 128-contraction with the (transposed, host-prepped)
weights stationary; fp8 DoubleRow everywhere except the fp16 static
branch (precision). Attention runs on 128-gene blocks (8 groups):
transposed scores [k, q] per head from K=64 matmuls (even heads in PE
rows 0:64, odd heads rows 64:128, separate PSUM banks), exp on scalar,
one [128,512] masked multiply per 4-head group on DVE, then attn @ V
with the ones-augmented gene-major V giving the output AND the softmax
denominator in one matmul; normalization is one reciprocal + one
broadcast multiply per group.  The gene-major attention output is
transposed back to feature-major with plain identity matmuls (full-rate
PE path, not transpose-mode).

The PE clock gate (HAM) re-throttles to half rate whenever PE duty in
its ~3.4us window drops below ~0.8, so the kernel is software-pipelined
to keep PE duty high through the attention stretches: the static branch
is emitted as filler inside layer-0 attention, and the NEXT superblock's
layer-0 Q/K projections are filler inside layer-1 attention (inputs are
prefetched one superblock ahead).

All biases except bq/bs are folded away on the host:
  - bk shifts scores uniformly along q -> softmax-invariant, dropped.
  - bv/bo fold into a per-layer constant c_l added to the residual
    stream (softmax rows sum to 1); the layer-1 constant solves
    (I + beta1 Wo1 Wv1) c1 = beta1 (Wo1 bv1 + bo1) so the shifted
    stream x~ = x + c feeds layer-1 Q/K/V directly with adjusted bq1.
  - The shifted x~0 also feeds the static branch with bs' = bs - Ws d.
The residual base is the same f16 feature-major x~0 the static branch
uses, so there is no fp32 x copy (or DMA) at all.
"""

import sys

sys.path.insert(0, "/opt/trn_rl_repo")

import numpy as np

import concourse.bacc as bacc
import concourse.mybir as mybir
from concourse import tile

H = 768
NT = H // 128  # 6 feature tiles
NH = 12
HD = 64
SB = 512  # genes per superblock
NBLK = SB // 128  # attention blocks per superblock
N_CORES = 8
N_GENES = 32768
GPC = N_GENES // N_CORES  # genes per core
F16 = mybir.dt.float16
F32 = mybir.dt.float32
F8 = mybir.dt.float8e4
DR = mybir.MatmulPerfMode.DoubleRow
AF = mybir.ActivationFunctionType

# aout column order: heads laid out slot-major so each pair-group's four
# heads occupy 256 contiguous columns (P = slot -> head).
PSLOT = (0, 2, 1, 3, 4, 6, 5, 7, 8, 10, 9, 11)


def build_program(gpc: int = GPC):
    nsb = gpc // SB
    nc = bacc.Bacc(None, target_bir_lowering=False)

    xt_h = nc.dram_tensor("xt_h", [H, gpc], F16, kind="ExternalInput")
    wsT = nc.dram_tensor("wsT", [H, H], F16, kind="ExternalInput")
    bs = nc.dram_tensor("bs", [H, 1], F32, kind="ExternalInput")
    xt_8 = nc.dram_tensor("xt_8", [H, gpc], F8, kind="ExternalInput")
    wq8 = nc.dram_tensor("wq8", [2, 3, 128, 2 * H], F8, kind="ExternalInput")
    wk8 = nc.dram_tensor("wk8", [2, 3, 128, 2 * H], F8, kind="ExternalInput")
    wv8 = nc.dram_tensor("wv8", [2, 3, 128, 2 * H], F8, kind="ExternalInput")
    wo8 = nc.dram_tensor("wo8", [2, 3, 128, 2 * H], F8, kind="ExternalInput")
    bq = nc.dram_tensor("bq", [2, H, 1], F32, kind="ExternalInput")
    mask4 = nc.dram_tensor("mask4", [128, SB], F16, kind="ExternalInput")
    ident = nc.dram_tensor("ident", [128, 128], F16, kind="ExternalInput")

    staticT = nc.dram_tensor("staticT", [H, gpc], F32, kind="ExternalOutput")
    outT = nc.dram_tensor("outT", [H, gpc], F32, kind="ExternalOutput")

    from contextlib import ExitStack

    with tile.TileContext(nc) as tc, ExitStack() as ctx:
        if True:
            wpool = ctx.enter_context(tc.tile_pool(name="wpool", bufs=1))
            xh_pool = ctx.enter_context(tc.tile_pool(name="xh", bufs=14))
            qk_pool = ctx.enter_context(tc.tile_pool(name="qk", bufs=26))
            v_pool = ctx.enter_context(tc.tile_pool(name="vaug", bufs=4))
            ea_pool = ctx.enter_context(tc.tile_pool(name="ea", bufs=4))
            aout_pool = ctx.enter_context(tc.tile_pool(name="aout", bufs=3))
            af_pool = ctx.enter_context(tc.tile_pool(name="af", bufs=7))
            x1_pool = ctx.enter_context(tc.tile_pool(name="x1", bufs=7))
            x8_pool = ctx.enter_context(tc.tile_pool(name="x8p", bufs=12))
            res_pool = ctx.enter_context(tc.tile_pool(name="res", bufs=3))
            small_pool = ctx.enter_context(tc.tile_pool(name="small", bufs=4))
            pbig = ctx.enter_context(tc.tile_pool(name="pbig", bufs=2, space="PSUM"))
            psc = ctx.enter_context(tc.tile_pool(name="psc", bufs=1, space="PSUM"))
            patt = ctx.enter_context(tc.tile_pool(name="patt", bufs=2, space="PSUM"))
            ptp = ctx.enter_context(tc.tile_pool(name="ptp", bufs=2, space="PSUM"))
            # ---- resident constants / weights ----
            ws_sb = []
            wq_sb = [[], []]
            wk_sb = [[], []]
            wv_sb = [[], []]
            wo_sb = [[], []]
            for k in range(NT):
                t = wpool.tile([128, H], F16, name=f"ws{k}", tag=f"ws{k}")
                nc.sync.dma_start(t[:], wsT[k * 128 : (k + 1) * 128, :])
                ws_sb.append(t)

            def load_layer_weights(l):
                for name, dram, lst in (
                    ("wq", wq8, wq_sb),
                    ("wk", wk8, wk_sb),
                    ("wv", wv8, wv_sb),
                    ("wo", wo8, wo_sb),
                ):
                    for kk in range(3):
                        t = wpool.tile(
                            [128, 2 * H], F8, name=f"{name}{l}{kk}", tag=f"{name}{l}{kk}"
                        )
                        nc.sync.dma_start(t[:], dram[l, kk])
                        lst[l].append(t)

            bs_t = wpool.tile([128, NT], F32, name="bs", tag="bs")
            bq_t = [wpool.tile([128, NT], F32, name=f"bq{l}", tag=f"bq{l}") for l in range(2)]
            nc.sync.dma_start(bs_t[:], bs[:, 0].rearrange("(m p) -> p m", p=128))
            for l in range(2):
                nc.sync.dma_start(bq_t[l][:], bq[l, :, 0].rearrange("(m p) -> p m", p=128))
            mask_t = wpool.tile([128, SB], F16, name="mask", tag="mask")
            nc.sync.dma_start(mask_t[:], mask4[:])
            ident_t = wpool.tile([128, 128], F16, name="ident", tag="ident")
            nc.sync.dma_start(ident_t[:], ident[:])
            ident8_t = wpool.tile([128, 128], F8, name="ident8", tag="ident8")
            nc.scalar.activation(ident8_t[:], ident_t[:], AF.Copy, scale=1.0)
            # 256*I: injects the residual stream into the o-proj PSUM, whose
            # weights are host-scaled by 2^8 to clear fp8's subnormal floor
            ident_sc = wpool.tile([128, 128], F16, name="identsc", tag="identsc")
            nc.scalar.activation(ident_sc[:], ident_t[:], AF.Copy, scale=256.0)

            def dma_sb(sb):
                g0 = sb * SB
                xh = []
                for k in range(NT):
                    t = xh_pool.tile([128, SB], F16, name="xh", tag="xh")
                    nc.sync.dma_start(t[:], xt_h[k * 128 : (k + 1) * 128, g0 : g0 + SB])
                    xh.append(t)
                x8 = []
                for kk in range(3):
                    t = x8_pool.tile([128, 2 * SB], F8, name="x8", tag="x8")
                    for j in range(2):
                        nc.sync.dma_start(
                            t[:, j * SB : (j + 1) * SB],
                            xt_8[(2 * kk + j) * 128 : (2 * kk + j + 1) * 128, g0 : g0 + SB],
                        )
                    x8.append(t)
                return xh, x8

            def qk_mm(ps, w_sb_l, xin_8, m):
                for kk in range(3):
                    nc.tensor.matmul(
                        ps[:],
                        w_sb_l[kk][:]
                        .rearrange("p (j o) -> p j o", o=H)[:, :, m * 128 : (m + 1) * 128],
                        xin_8[kk][:].rearrange("p (j n) -> p j n", n=SB),
                        start=(kk == 0),
                        stop=(kk == 2),
                        perf_mode=DR,
                    )

            def qk_finish(dest, m, ps, b_t):
                qt = qk_pool.tile([128, SB], F8, name="qk", tag="qk")
                if b_t is not None:
                    nc.vector.tensor_scalar_add(qt[:], ps[:], b_t[:, m : m + 1])
                elif m % 2 == 0:
                    nc.vector.tensor_copy(qt[:], ps[:])
                else:
                    nc.scalar.activation(qt[:], ps[:], AF.Copy, scale=1.0)
                dest[m] = qt

            def project_qk(l, xin_8):
                q_tiles, k_tiles = [None] * NT, [None] * NT
                for dest, w_sb_l, b_t in (
                    (q_tiles, wq_sb[l], bq_t[l]),
                    (k_tiles, wk_sb[l], None),
                ):
                    for m in range(NT):
                        ps = pbig.tile([128, SB], F32, name="pbig", tag="pbig")
                        qk_mm(ps, w_sb_l, xin_8, m)
                        qk_finish(dest, m, ps, b_t)
                return q_tiles, k_tiles

            def make_static_filler(m, xh_tiles, g0):
                def f():
                    ps = pbig.tile([128, SB], F32, name="pbig", tag="pbig")
                    for k in range(NT):
                        nc.tensor.matmul(
                            ps[:],
                            ws_sb[k][:, m * 128 : (m + 1) * 128],
                            xh_tiles[k][:],
                            start=(k == 0),
                            stop=(k == NT - 1),
                        )
                    st = res_pool.tile([128, SB], F32, name="st", tag="st")
                    nc.scalar.activation(
                        st[:], ps[:], AF.Relu, bias=bs_t[:, m : m + 1], scale=1.0
                    )
                    nc.sync.dma_start(
                        staticT[m * 128 : (m + 1) * 128, g0 : g0 + SB], st[:]
                    )
                return f

            def make_qk_filler(dest, w_sb_l, b_t, xin_8, m):
                def f():
                    ps = pbig.tile([128, SB], F32, name="pbig", tag="pbig")
                    qk_mm(ps, w_sb_l, xin_8, m)
                    qk_finish(dest, m, ps, b_t)
                return f

            def attention_block(l, q_tiles, k_tiles, xin_8, blk, af8_tiles, fillers):
                """V-proj + group-local attention for one 128-gene block,
                staged so the PE queue never blocks on a long cross-engine
                chain; pops up to 3 filler closures into the stalls."""
                b0 = blk * 128

                def fill():
                    if fillers:
                        fillers.pop(0)()

                e_ts = [None] * 3
                a_ts = [None] * 3

                def scores(g):
                    hA = (4 * g, 4 * g + 2)
                    hB = (4 * g + 1, 4 * g + 3)
                    ps_lo = psc.tile([128, 256], F32, name="psl", tag="psl")
                    ps_hi = psc.tile([128, 256], F32, name="psh", tag="psh")
                    for j in range(2):
                        nc.tensor.matmul(
                            ps_lo[:, j * 128 : (j + 1) * 128],
                            k_tiles[hA[j] // 2][0:HD, b0 : b0 + 128],
                            q_tiles[hA[j] // 2][0:HD, b0 : b0 + 128],
                            start=True, stop=True, tile_position=(0, 0),
                        )
                    for j in range(2):
                        nc.tensor.matmul(
                            ps_hi[:, j * 128 : (j + 1) * 128],
                            k_tiles[hB[j] // 2][HD:128, b0 : b0 + 128],
                            q_tiles[hB[j] // 2][HD:128, b0 : b0 + 128],
                            start=True, stop=True, tile_position=(HD, 0),
                        )
                    # q/k are host-scaled by 16 each -> fold 1/256 into exp
                    e_t = ea_pool.tile([128, 512], F16, name="e", tag="e")
                    nc.scalar.activation(e_t[:, 0:256], ps_lo[:], AF.Exp, scale=0.125 / 256.0)
                    nc.scalar.activation(e_t[:, 256:512], ps_hi[:], AF.Exp, scale=0.125 / 256.0)
                    a_t = ea_pool.tile([128, 512], F16, name="a", tag="a")
                    nc.gpsimd.tensor_mul(a_t[:], e_t[:], mask_t[:])
                    e_ts[g], a_ts[g] = e_t, a_t

                def av(g, va):
                    ps_o = patt.tile([128, 4 * (HD + 1)], F32, name="patt", tag="patt")
                    po = ps_o[:].rearrange("p (j c) -> p j c", c=HD + 1)
                    for j in range(4):
                        nc.tensor.matmul(
                            ps_o[:, j * (HD + 1) : (j + 1) * (HD + 1)],
                            a_ts[g][:, j * 128 : (j + 1) * 128],
                            va[:, 4 * g + j, :],
                            start=True, stop=True,
                        )
                    r4 = small_pool.tile([128, 4], F32, name="r4", tag="r4")
                    nc.vector.reciprocal(r4[:], po[:, :, HD])
                    nc.vector.tensor_mul(
                        aout[:, g * 256 : (g + 1) * 256].rearrange(
                            "p (j c) -> p j c", c=HD
                        ),
                        po[:, :, 0:HD],
                        r4[:].broadcast_to([128, 4, HD]),
                    )

                def tp(g):
                    ps_t = ptp.tile([128, 256], F32, name="ptp", tag="ptp")
                    for i, t in enumerate((2 * g, 2 * g + 1)):
                        nc.tensor.matmul(
                            ps_t[:, i * 128 : (i + 1) * 128],
                            aout[:, t * 128 : (t + 1) * 128],
                            ident8_t[:],
                            start=True,
                            stop=True,
                        )
                    dst = (
                        af8_tiles[g][:]
                        .rearrange("p (j n) -> p j n", n=SB)[:, :, b0 : b0 + 128]
                    )
                    src = ps_t[:].rearrange("p (j n) -> p j n", n=128)
                    if g % 2 == 0:
                        nc.vector.tensor_copy(dst, src)
                    else:
                        nc.scalar.activation(dst, src, AF.Copy, scale=1.0)

                scores(0)
                # V projection, gene-major with interleaved ones columns
                # (slot-major head order baked into wv8 on the host)
                vaug = v_pool.tile([128, NH * (HD + 1)], F16, name="vaug", tag="vaug")
                va = vaug[:].rearrange("p (h c) -> p h c", c=HD + 1)
                for half in range(2):
                    psv = pbig.tile([128, 384], F32, name="pbig", tag="pbig")
                    for kk in range(3):
                        nc.tensor.matmul(
                            psv[:],
                            xin_8[kk][:]
                            .rearrange("p (j n) -> p j n", n=SB)[
                                :, :, b0 : b0 + 128
                            ],
                            wv_sb[l][kk][:]
                            .rearrange("p (j o) -> p j o", o=H)[
                                :, :, half * 384 : (half + 1) * 384
                            ],
                            start=(kk == 0),
                            stop=(kk == 2),
                            perf_mode=DR,
                        )
                    if half == 0:
                        nc.vector.tensor_copy(
                            va[:, 0:6, 0:HD],
                            psv[:].rearrange("p (h c) -> p h c", c=HD),
                        )
                    else:
                        nc.scalar.activation(
                            va[:, 6:12, 0:HD],
                            psv[:].rearrange("p (h c) -> p h c", c=HD),
                            AF.Copy,
                            scale=1.0,
                        )
                # v is host-scaled by 16; a 16.0 ones column makes the
                # denominator carry the same factor, so normalization
                # cancels the scale automatically
                nc.vector.memset(va[:, :, HD : HD + 1], 16.0)
                aout = aout_pool.tile([128, H], F8, name="aout", tag="aout")
                scores(1)
                fill()
                scores(2)
                av(0, va)
                av(1, va)
                fill()
                av(2, va)
                tp(0)
                tp(1)
                fill()
                tp(2)

            def o_proj(l, af8_tiles, xin_f, g0):
                # wo8 is host-scaled by 2^8 (else beta*Wo flushes to zero in
                # fp8); the residual rides in as 256*I @ x~ and the PSUM
                # drain applies the 2^-8.
                new_f = []
                for m in range(NT):
                    ps = pbig.tile([128, SB], F32, name="pbig", tag="pbig")
                    for kk in range(3):
                        nc.tensor.matmul(
                            ps[:],
                            wo_sb[l][kk][:]
                            .rearrange("p (j o) -> p j o", o=H)[
                                :, :, m * 128 : (m + 1) * 128
                            ],
                            af8_tiles[kk][:].rearrange("p (j n) -> p j n", n=SB),
                            start=(kk == 0),
                            stop=False,
                            perf_mode=DR,
                        )
                    nc.tensor.matmul(
                        ps[:], ident_sc[:], xin_f[m][:], start=False, stop=True
                    )
                    if l == 0:
                        xnf = x1_pool.tile([128, SB], F16, name="x1f", tag="x1f")
                        nc.vector.tensor_scalar_mul(xnf[:], ps[:], 1.0 / 256.0)
                        new_f.append(xnf)
                    else:
                        xo = res_pool.tile([128, SB], F32, name="xo", tag="xo")
                        nc.vector.tensor_scalar_mul(xo[:], ps[:], 1.0 / 256.0)
                        nc.sync.dma_start(
                            outT[m * 128 : (m + 1) * 128, g0 : g0 + SB], xo[:]
                        )
                return new_f

            # ---------------- main superblock loop ----------------
            qk_next = None
            xh_cur = x8_cur = None
            for sb in range(nsb):
                g0 = sb * SB
                first_static = 0
                if sb == 0:
                    xh_cur, x8_cur = dma_sb(0)
                    load_layer_weights(0)
                    load_layer_weights(1)
                    # two static tiles first: they only need ws+xh, so the
                    # PE starts (and HAM warms) while wq/wk still stream in
                    for m in range(2):
                        make_static_filler(m, xh_cur, g0)()
                    first_static = 2
                    q0, k0 = project_qk(0, x8_cur)
                else:
                    q0, k0 = qk_next
                if sb + 1 < nsb:
                    xh_nxt, x8_nxt = dma_sb(sb + 1)
                else:
                    xh_nxt = x8_nxt = None

                # layer 0: attention with the static branch as PE filler
                fillers0 = []
                for m in range(first_static, NT):
                    fillers0.append(make_static_filler(m, xh_cur, g0))
                af8_a = [
                    af_pool.tile([128, 2 * SB], F8, name="af", tag="af")
                    for _ in range(3)
                ]
                for blk in range(NBLK):
                    attention_block(0, q0, k0, x8_cur, blk, af8_a, fillers0)
                while fillers0:
                    fillers0.pop(0)()
                new_f = o_proj(0, af8_a, xh_cur, g0)
                x8_1 = []
                for kk in range(3):
                    t = x8_pool.tile([128, 2 * SB], F8, name="x81", tag="x8")
                    x8_1.append(t)
                for m in range(NT):
                    kk, j = divmod(m, 2)
                    nc.gpsimd.tensor_copy(
                        x8_1[kk][:, j * SB : (j + 1) * SB], new_f[m][:]
                    )

                # layer 1: attention with next superblock's layer-0 Q/K
                # projections as PE filler
                q1, k1 = project_qk(1, x8_1)
                fillers1 = []
                if x8_nxt is not None:
                    qn, kn = [None] * NT, [None] * NT
                    for m in range(NT):
                        fillers1.append(
                            make_qk_filler(qn, wq_sb[0], bq_t[0], x8_nxt, m)
                        )
                        fillers1.append(
                            make_qk_filler(kn, wk_sb[0], None, x8_nxt, m)
                        )
                    qk_next = (qn, kn)
                af8_b = [
                    af_pool.tile([128, 2 * SB], F8, name="af", tag="af")
                    for _ in range(3)
                ]
                for blk in range(NBLK):
                    attention_block(1, q1, k1, x8_1, blk, af8_b, fillers1)
                while fillers1:
                    fillers1.pop(0)()
                o_proj(1, af8_b, new_f, g0)
                xh_cur, x8_cur = xh_nxt, x8_nxt

    nc.finalize()
    return nc


def host_prep(inputs: dict, core: int, gpc: int = GPC) -> dict:
    """Slice/transpose/cast inputs for one core."""
    ge = np.asarray(inputs["gene_embeddings"], np.float32)
    Ws = np.asarray(inputs["W_static"], np.float32)
    bs = np.asarray(inputs["b_static"], np.float32)
    Wq = np.asarray(inputs["Wq"], np.float64)
    bq = np.asarray(inputs["bq"], np.float64)
    Wk = np.asarray(inputs["Wk"], np.float64)
    Wv = np.asarray(inputs["Wv"], np.float64)
    bv = np.asarray(inputs["bv"], np.float64)
    Wo = np.asarray(inputs["Wo"], np.float64)
    bo = np.asarray(inputs["bo"], np.float64)
    beta = np.asarray(inputs["beta"], np.float64)

    # fold bv/bo into a constant shift of the residual stream (see module
    # docstring): d0 open-loop for layer 0, c1 as a fixpoint for layer 1.
    d0 = beta[0] * (Wo[0] @ bv[0] + bo[0])
    c1 = np.linalg.solve(
        np.eye(H) + beta[1] * (Wo[1] @ Wv[1]), beta[1] * (Wo[1] @ bv[1] + bo[1])
    )
    d = (d0 + c1).astype(np.float32)
    bq_dev = (16.0 * np.stack([bq[0], bq[1] - Wq[1] @ c1])).astype(np.float32)
    bs_dev = (bs.astype(np.float64) - Ws.astype(np.float64) @ d).astype(np.float32)

    xs = ge[core * gpc : (core + 1) * gpc].T  # [768, gpc]
    f8 = mybir.dt.np(F8)

    # head permutation for V-out / Wo-in feature columns (slot-major)
    perm = np.concatenate([np.arange(HD) + p * HD for p in PSLOT])

    def pack8(WT):  # [2, H, H] (k, o) -> [2, 3, 128, 2H] DoubleRow pairs
        out = np.empty((2, 3, 128, 2 * H), f8)
        for l in range(2):
            for kk in range(3):
                for j in range(2):
                    out[l, kk, :, j * H : (j + 1) * H] = WT[
                        l, (2 * kk + j) * 128 : (2 * kk + j + 1) * 128, :
                    ].astype(f8)
        return out

    # power-of-2 upscaling clears fp8e4m3's subnormal floor (~2e-3): q/k by
    # 16 (compensated in the exp scale), v by 16 (compensated by the 16.0
    # ones column via the softmax denominator), beta*Wo by 256 (compensated
    # at the o-proj PSUM drain).
    wqT_f = (16.0 * Wq.transpose(0, 2, 1)).astype(np.float32)
    wkT_f = (16.0 * Wk.transpose(0, 2, 1)).astype(np.float32)
    wvT_f = (16.0 * Wv.transpose(0, 2, 1)[:, :, perm]).astype(np.float32)
    woT_f = np.stack([256.0 * beta[l] * Wo[l].T for l in range(2)])[
        :, perm, :
    ].astype(np.float32)
    # block-diagonal (8 groups of 16) minus identity, tiled 4x
    m = np.kron(np.eye(8, dtype=np.float16), np.ones((16, 16), np.float16))
    m -= np.eye(128, dtype=np.float16)
    mask4 = np.tile(m, (1, 4))
    return {
        "xt_h": np.ascontiguousarray(xs + d[:, None], dtype=np.float16),
        "wsT": np.ascontiguousarray(Ws.T, dtype=np.float16),
        "bs": bs_dev.reshape(H, 1),
        "xt_8": np.ascontiguousarray(xs).astype(f8),
        "wq8": pack8(wqT_f),
        "wk8": pack8(wkT_f),
        "wv8": pack8(wvT_f),
        "wo8": pack8(woT_f),
        "bq": bq_dev.reshape(2, H, 1),
        "mask4": np.ascontiguousarray(mask4),
        "ident": np.eye(128, dtype=np.float16),
    }


_CACHED = {}


def _get_program():
    if "nc" not in _CACHED:
        _CACHED["nc"] = build_program(GPC)
    return _CACHED["nc"]


def kernel(**inputs):
    from concourse.bass_utils import run_bass_kernel_spmd

    nc = _get_program()
    in_maps = [host_prep(inputs, c) for c in range(N_CORES)]
    res = run_bass_kernel_spmd(nc, in_maps, list(range(N_CORES)))
    static = np.concatenate([np.asarray(r["staticT"]).T for r in res.results], axis=0)
    x = np.concatenate([np.asarray(r["outT"]).T for r in res.results], axis=0)
    return static.astype(np.float32), x.astype(np.float32)


if __name__ == "__main__":
    nc = build_program(GPC)
    print("build ok")
